# revision 49
# baseline (speedup 1.0000x reference)
"""Causal multi-head attention (b=2, h=32, s=2048, d=128, fp32) on 8 TRN2 NeuronCores.

Sharding: the 64 (batch, head) pairs are split 8-per-core (tensor parallel over
heads); each core runs an identical Bass/Tile kernel on its own heads.

Per-core kernel, S^T formulation with a q-chunked PV and a free softmax
denominator:
  Q^T/K^T/V/P are all bf16 (1 cycle/row matmuls at any width; halves DMA).
  The 40 causal S^T tile-segments per head are packed (widest-fit-decreasing,
  PSUM-bank-aligned) into groups alternating 1536/1024 columns, so each exp
  instruction covers ~1280 columns and the scalar engine's ~285ns/instruction
  overhead is amortized over ~109 instructions.  P^T = exp(S^T/sqrt(d)), no
  max-subtraction needed.  Diagonal-tile triangles are zeroed by DVE
  multiplies with a constant 0/1 triangle, two fused per op via a strided AP.
  PV is computed per 128-wide q-chunk with the P^T chunk as the STATIONARY
  and V extended by a ones-column as the moving operand:
      ctx[q, 0:128] , l[q] = sum_k P^T[k, q] * [V | 1][k, :]
  so the softmax denominator costs one extra matmul column (+0.8% PE) and the
  entire DVE/gpsimd l-accumulation of earlier designs disappears (power
  throttling punishes multi-engine designs: wall ~ total-engine-work/2.45).
  ctx lands q-major, so the epilogue is a [128,1] reciprocal and a 128-col
  broadcast multiply per chunk on the DVE, and chunks retire as soon as their
  diagonal tile is done - well before block end.  PSUM: 3+2 banks for the two
  S group buffers, 3 banks of [128, 3x129] ctx+l chunk tiles.
"""
import math
import sys

if '/opt/trn_rl_repo' not in sys.path:
    sys.path.insert(0, '/opt/trn_rl_repo')

import numpy as np
import ml_dtypes

import concourse.bass as bass
import concourse.tile as tile
from concourse import mybir, bacc
from concourse.bass_utils import run_bass_kernel_spmd

F32 = mybir.dt.float32
BF16 = mybir.dt.bfloat16
EXP = mybir.ActivationFunctionType.Exp
MULT = mybir.AluOpType.mult
ADD = mybir.AluOpType.add

B, H, S, D = 2, 32, 2048, 128
N_CORES = 8
HPC = (B * H) // N_CORES     # (b,h) pairs per core
QB = 512                     # q-block width (S matmul granularity)
NQB = S // QB
NKT = S // 128               # k-tiles per head
DE = D + 1                   # V extended with a ones column
SCALE = 1.0 / math.sqrt(D)


def _plan(n_heads):
    """Pack all (head, q-block, k-tile) S^T segments into PSUM groups with
    capacities alternating 1536/1024 columns (widest-fit-decreasing inside a
    block; 512-wide segs front-fill banks, narrower back-fill so closed
    groups stay contiguous).  Each segment: pos = column inside the group."""
    groups = []
    cur, banks = [], [0] * 3

    def new_group():
        nonlocal cur, banks
        nbanks = 3 if len(groups) % 2 == 0 else 2
        cur, banks = [], [0] * nbanks

    new_group()
    for h in range(n_heads):
        for j in range(NQB):
            rem = []
            for t in range(4 * j + 4):    # causal: k-tiles 0..4j+3
                o = max(t - 4 * j, 0) * 128
                rem.append(dict(h=h, j=j, t=t, w=QB - o, o=o))
            while rem:
                placed = None
                for sg in sorted(rem, key=lambda s: -s["w"]):
                    w = sg["w"]
                    rng = (range(len(banks)) if w == 512
                           else range(len(banks) - 1, -1, -1))
                    for b in rng:
                        if banks[b] + w <= 512:
                            sg["pos"] = b * 512 + banks[b]
                            banks[b] += w
                            placed = sg
                            break
                    if placed:
                        break
                if placed is None:
                    groups.append(cur)
                    new_group()
                    continue
                rem.remove(placed)
                cur.append(placed)
    if cur:
        groups.append(cur)

    # a q-chunk b of block j is complete once every tile t <= b has been
    # placed; with FFD reordering that isn't necessarily at seg (j, t=b),
    # so compute the completing segment for each chunk explicitly
    placed = {}
    for gi, grp in enumerate(groups):
        for si, sg in enumerate(grp):
            sg["chunks_done"] = []
            placed[(sg["h"], sg["j"], sg["t"])] = (gi, si, sg)
    for h in {k[0] for k in placed}:
        for j in range(NQB):
            for b in range(4 * j, 4 * j + 4):
                last = max((placed[(h, j, t)] for t in range(b + 1)),
                           key=lambda x: (x[0], x[1]))
                last[2]["chunks_done"].append((h, j, b))

    # contiguous written-column runs per group (exp must not read packing
    # holes - they hold a previous tile's stale data)
    out = []
    for grp in groups:
        ivs = sorted((sg["pos"], sg["pos"] + sg["w"]) for sg in grp)
        runs = [list(ivs[0])]
        for a, b in ivs[1:]:
            if a == runs[-1][1]:
                runs[-1][1] = b
            else:
                runs.append([a, b])
        out.append(dict(segs=grp, runs=runs))
    return out


def _build(n_heads=HPC, la=3, p_bufs=10, ctx_bufs=3):
    nc = bacc.Bacc("TRN2", target_bir_lowering=False, debug=False,
                   num_devices=N_CORES)
    qt = nc.dram_tensor("qt", [n_heads, 128, S], BF16, kind="ExternalInput")
    kt = nc.dram_tensor("kt", [n_heads, 128, S], BF16, kind="ExternalInput")
    # v layout [h, k_local(128), k_tile(16), DE]; last feature column is 1.0
    v = nc.dram_tensor("v", [n_heads, 128, NKT, DE], BF16, kind="ExternalInput")
    # tri[r, c] = 1 where c >= r else 0 (causal keep-triangle)
    tri = nc.dram_tensor("tri", [128, 128], BF16, kind="ExternalInput")
    # output is q-major per head
    out = nc.dram_tensor("out", [n_heads, S, D], BF16, kind="ExternalOutput")

    groups = _plan(n_heads)

    with tile.TileContext(nc) as tc:
        with (tc.tile_pool(name="heads", bufs=2) as hp,
              tc.tile_pool(name="consts", bufs=1) as cp,
              tc.tile_pool(name="pp", bufs=p_bufs) as pp,
              tc.tile_pool(name="outp", bufs=4) as outp,
              tc.tile_pool(name="ps_s", bufs=1, space="PSUM") as ps_s,
              tc.tile_pool(name="ps_c", bufs=ctx_bufs, space="PSUM") as ps_c):
            tri_sb = cp.tile([128, 128], BF16)
            nc.sync.dma_start(tri_sb, tri[:, :])

            head_sb = {}     # h -> (qt_sb, kt_sb, v_sb)
            blk_segs = {}    # (h, j) -> {t: (p_sb, pos, o)}

            def prep_head(h):
                if h in head_sb:
                    return head_sb[h]
                qt_sb = hp.tile([128, S], BF16, tag="qt", name="qt_sb")
                kt_sb = hp.tile([128, S], BF16, tag="kt", name="kt_sb")
                v_sb = hp.tile([128, NKT, DE], BF16, tag="v", name="v_sb")
                for c0 in range(0, S, 512):
                    nc.sync.dma_start(kt_sb[:, c0:c0 + 512], kt[h, :, c0:c0 + 512])
                    nc.sync.dma_start(qt_sb[:, c0:c0 + 512], qt[h, :, c0:c0 + 512])
                for t0 in range(0, NKT, 4):
                    nc.sync.dma_start(v_sb[:, t0:t0 + 4, :], v[h, :, t0:t0 + 4, :])
                head_sb[h] = (qt_sb, kt_sb, v_sb)
                return head_sb[h]

            def emit_s(gi, grp):
                if gi % 2 == 0:
                    s_ps = ps_s.tile([128, 1536], F32, tag="sA", name="s_psA")
                else:
                    s_ps = ps_s.tile([128, 1024], F32, tag="sB", name="s_psB")
                for sg in grp["segs"]:
                    qt_sb, kt_sb, _ = prep_head(sg["h"])
                    t, j = sg["t"], sg["j"]
                    nc.tensor.matmul(
                        s_ps[:, sg["pos"]:sg["pos"] + sg["w"]],
                        kt_sb[:, t * 128:(t + 1) * 128],
                        qt_sb[:, j * QB + sg["o"]:(j + 1) * QB],
                        start=True, stop=True)
                return s_ps

            def emit_chunk_pv(h, j, b):
                # q-chunk b's P tiles (t = 0..b) all exist: one serial burst
                # of [V|1]-moving matmuls into its own PSUM bank (only one
                # open accumulation group per bank is legal), epilogue
                # immediately after.
                segs = blk_segs[(h, j)]
                _, _, v_sb = head_sb[h]
                ck = ps_c.tile([128, DE], F32, tag="ctx", name="ctx_ps")
                for t in range(b + 1):
                    p_sb, pos, o = segs[t]
                    cpos = pos + (b - 4 * j) * 128 - o
                    nc.tensor.matmul(ck, p_sb[:, cpos:cpos + 128],
                                     v_sb[:, t, :],
                                     start=(t == 0), stop=(t == b))
                recip_sb = outp.tile([128, 1], F32, tag="recip",
                                     name="recip_sb")
                nc.vector.reciprocal_approx_fast(recip_sb, ck[:, D:DE])
                ctx_sb = outp.tile([128, D], BF16, tag="ctx_out",
                                   name="ctx_sb")
                nc.vector.tensor_scalar_mul(ctx_sb, ck[:, 0:D], recip_sb)
                nc.sync.dma_start(out[h, b * 128:(b + 1) * 128, :], ctx_sb)
                if b == 4 * j + 3:
                    del blk_segs[(h, j)]

            pending = [emit_s(gi, g) for gi, g in enumerate(groups[:la])]
            for i, grp in enumerate(groups):
                s_ps = pending.pop(0)

                p_sb = pp.tile([128, 1536], BF16, tag="p", name="p_sb")
                for a, bnd in grp["runs"]:
                    nc.scalar.activation(p_sb[:, a:bnd], s_ps[:, a:bnd], EXP,
                                         scale=SCALE)

                # zero the below-diagonal triangles, two per DVE op
                diag = sorted((sg["pos"] for sg in grp["segs"]
                               if sg["t"] >= 4 * sg["j"]))
                while diag:
                    if len(diag) >= 2:
                        p0, p1 = diag.pop(0), diag.pop(0)
                        pap = bass.AP(tensor=p_sb.tensor,
                                      offset=p_sb.offset + p0,
                                      ap=[p_sb.ap[0], [p1 - p0, 2], [1, 128]])
                        tap = bass.AP(tensor=tri_sb.tensor,
                                      offset=tri_sb.offset,
                                      ap=[tri_sb.ap[0], [0, 2], [1, 128]])
                        nc.vector.tensor_tensor(out=pap, in0=pap, in1=tap,
                                                op=MULT)
                    else:
                        p0 = diag.pop(0)
                        nc.vector.tensor_tensor(
                            out=p_sb[:, p0:p0 + 128],
                            in0=p_sb[:, p0:p0 + 128],
                            in1=tri_sb, op=MULT)

                done = []
                for sg in grp["segs"]:
                    blk_segs.setdefault((sg["h"], sg["j"]), {})[sg["t"]] = (
                        p_sb, sg["pos"], sg["o"])
                    done.extend(sg["chunks_done"])
                for h, j, b in done:
                    emit_chunk_pv(h, j, b)

                # emit the lookahead S group AFTER this group's PV bursts so
                # its s-ring WAR wait (on exp of group i+1) doesn't head-of-
                # line block the bursts in the in-order PE queue
                if i + la < len(groups):
                    pending.append(emit_s(i + la, groups[i + la]))

    nc.compile()
    return nc


_NC_CACHE = None


def _get_nc():
    global _NC_CACHE
    if _NC_CACHE is None:
        _NC_CACHE = _build()
    return _NC_CACHE


def _prep_inputs(q, k, v):
    """Full [b,h,s,d] f32 inputs -> per-core bf16 input maps."""
    bf = ml_dtypes.bfloat16
    qf = np.asarray(q, np.float32).reshape(B * H, S, D)
    kf = np.asarray(k, np.float32).reshape(B * H, S, D)
    vf = np.asarray(v, np.float32).reshape(B * H, S, D)
    qt = qf.transpose(0, 2, 1).astype(bf)                    # [64, d, s]
    kt = kf.transpose(0, 2, 1).astype(bf)
    # [64, k_local(128), t(16), D+1] with ones in the last feature column
    vr = np.ones((B * H, 128, NKT, DE), dtype=bf)
    vr[:, :, :, :D] = vf.reshape(B * H, NKT, 128, D).transpose(0, 2, 1, 3)
    tri_np = (np.arange(128)[None, :] >= np.arange(128)[:, None]).astype(bf)
    in_maps = []
    for c in range(N_CORES):
        sl = slice(c * HPC, (c + 1) * HPC)
        in_maps.append({
            "qt": np.ascontiguousarray(qt[sl]),
            "kt": np.ascontiguousarray(kt[sl]),
            "v": np.ascontiguousarray(vr[sl]),
            "tri": tri_np,
        })
    return in_maps


def kernel(query_layer, key_layer, value_layer, attention_mask):
    """Full-input causal attention; returns [b, s, h*d] float32."""
    # attention_mask is the standard causal mask (True = masked); the kernel
    # hardcodes causal masking, so the mask tensor itself is not shipped.
    in_maps = _prep_inputs(query_layer, key_layer, value_layer)
    nc = _get_nc()
    res = run_bass_kernel_spmd(nc, in_maps, core_ids=list(range(N_CORES)))

    # [64(bh), s, d] bf16 -> out[b, s, h*D+d] f32
    o_all = np.concatenate([res.results[c]["out"] for c in range(N_CORES)],
                           axis=0)
    return np.ascontiguousarray(
        o_all.astype(np.float32).reshape(B, H, S, D).transpose(0, 2, 1, 3)
    ).reshape(B, S, H * D)


# revision 52
# speedup vs baseline: 1.1214x; 1.1214x over previous
"""Causal multi-head attention (b=2, h=32, s=2048, d=128, fp32) on 8 TRN2 NeuronCores.

Sharding: the 64 (batch, head) pairs are split 8-per-core (tensor parallel over
heads); each core runs an identical Bass/Tile kernel on its own heads.

Per-core kernel, S^T formulation with a q-chunked PV and a free softmax
denominator:
  Q^T/K^T/V/P are all bf16 (1 cycle/row matmuls at any width; halves DMA).
  The 40 causal S^T tile-segments per head are packed (widest-fit-decreasing,
  PSUM-bank-aligned) into groups alternating 1536/1024 columns, so each exp
  instruction covers ~1280 columns and the scalar engine's ~285ns/instruction
  overhead is amortized over ~109 instructions.  P^T = exp(S^T/sqrt(d)), no
  max-subtraction needed.  Diagonal-tile triangles are zeroed by DVE
  multiplies with a constant 0/1 triangle, two fused per op via a strided AP.
  PV is computed per 128-wide q-chunk with the P^T chunk as the STATIONARY
  and V extended by a ones-column as the moving operand:
      ctx[q, 0:128] , l[q] = sum_k P^T[k, q] * [V | 1][k, :]
  so the softmax denominator costs one extra matmul column (+0.8% PE) and the
  entire DVE/gpsimd l-accumulation of earlier designs disappears (power
  throttling punishes multi-engine designs: wall ~ total-engine-work/2.45).
  ctx lands q-major, so the epilogue is a [128,1] reciprocal and a 128-col
  broadcast multiply per chunk on the DVE, and chunks retire as soon as their
  diagonal tile is done - well before block end.  PSUM: 3+2 banks for the two
  S group buffers, 3 banks of [128, 3x129] ctx+l chunk tiles.
"""
import math
import sys

if '/opt/trn_rl_repo' not in sys.path:
    sys.path.insert(0, '/opt/trn_rl_repo')

import numpy as np
import ml_dtypes

import concourse.bass as bass
import concourse.tile as tile
from concourse import mybir, bacc
from concourse.bass_utils import run_bass_kernel_spmd

F32 = mybir.dt.float32
BF16 = mybir.dt.bfloat16
EXP = mybir.ActivationFunctionType.Exp
MULT = mybir.AluOpType.mult
ADD = mybir.AluOpType.add

B, H, S, D = 2, 32, 2048, 128
N_CORES = 8
HPC = (B * H) // N_CORES     # (b,h) pairs per core
QB = 512                     # q-block width (S matmul granularity)
NQB = S // QB
NKT = S // 128               # k-tiles per head
DE = D + 1                   # V extended with a ones column
SCALE = 1.0 / math.sqrt(D)


def _plan(n_heads):
    """Pack all (head, q-block, k-tile) S^T segments into PSUM groups with
    capacities alternating 1536/1024 columns (widest-fit-decreasing inside a
    block; 512-wide segs front-fill banks, narrower back-fill so closed
    groups stay contiguous).  Each segment: pos = column inside the group."""
    groups = []
    cur, banks = [], [0] * 3

    def new_group():
        nonlocal cur, banks
        nbanks = 3 if len(groups) % 2 == 0 else 2
        cur, banks = [], [0] * nbanks

    new_group()
    for h in range(n_heads):
        for j in range(NQB):
            rem = []
            for t in range(4 * j + 4):    # causal: k-tiles 0..4j+3
                o = max(t - 4 * j, 0) * 128
                rem.append(dict(h=h, j=j, t=t, w=QB - o, o=o))
            while rem:
                placed = None
                for sg in sorted(rem, key=lambda s: -s["w"]):
                    w = sg["w"]
                    rng = (range(len(banks)) if w == 512
                           else range(len(banks) - 1, -1, -1))
                    for b in rng:
                        if banks[b] + w <= 512:
                            sg["pos"] = b * 512 + banks[b]
                            banks[b] += w
                            placed = sg
                            break
                    if placed:
                        break
                if placed is None:
                    groups.append(cur)
                    new_group()
                    continue
                rem.remove(placed)
                cur.append(placed)
    if cur:
        groups.append(cur)

    # a q-chunk b of block j is complete once every tile t <= b has been
    # placed; with FFD reordering that isn't necessarily at seg (j, t=b),
    # so compute the completing segment for each chunk explicitly
    placed = {}
    for gi, grp in enumerate(groups):
        for si, sg in enumerate(grp):
            sg["chunks_done"] = []
            placed[(sg["h"], sg["j"], sg["t"])] = (gi, si, sg)
    for h in {k[0] for k in placed}:
        for j in range(NQB):
            for b in range(4 * j, 4 * j + 4):
                last = max((placed[(h, j, t)] for t in range(b + 1)),
                           key=lambda x: (x[0], x[1]))
                last[2]["chunks_done"].append((h, j, b))

    # contiguous written-column runs per group (exp must not read packing
    # holes - they hold a previous tile's stale data)
    out = []
    for grp in groups:
        ivs = sorted((sg["pos"], sg["pos"] + sg["w"]) for sg in grp)
        runs = [list(ivs[0])]
        for a, b in ivs[1:]:
            if a == runs[-1][1]:
                runs[-1][1] = b
            else:
                runs.append([a, b])
        out.append(dict(segs=grp, runs=runs))
    return out


def _build(n_heads=HPC, la=2, p_bufs=10, ctx_bufs=3):
    nc = bacc.Bacc("TRN2", target_bir_lowering=False, debug=False,
                   num_devices=N_CORES)
    qt = nc.dram_tensor("qt", [n_heads, 128, S], BF16, kind="ExternalInput")
    kt = nc.dram_tensor("kt", [n_heads, 128, S], BF16, kind="ExternalInput")
    # v layout [h, k_local(128), k_tile(16), DE]; last feature column is 1.0
    v = nc.dram_tensor("v", [n_heads, 128, NKT, DE], BF16, kind="ExternalInput")
    # tri[r, c] = 1 where c >= r else 0 (causal keep-triangle)
    tri = nc.dram_tensor("tri", [128, 128], BF16, kind="ExternalInput")
    # output is q-major per head
    out = nc.dram_tensor("out", [n_heads, S, D], BF16, kind="ExternalOutput")

    groups = _plan(n_heads)

    with tile.TileContext(nc) as tc:
        with (tc.tile_pool(name="heads", bufs=2) as hp,
              tc.tile_pool(name="consts", bufs=1) as cp,
              tc.tile_pool(name="pp", bufs=p_bufs) as pp,
              tc.tile_pool(name="outp", bufs=4) as outp,
              tc.tile_pool(name="ps_s", bufs=1, space="PSUM") as ps_s,
              tc.tile_pool(name="ps_c", bufs=ctx_bufs, space="PSUM") as ps_c):
            tri_sb = cp.tile([128, 128], BF16)
            nc.sync.dma_start(tri_sb, tri[:, :])

            head_sb = {}     # h -> (qt_sb, kt_sb, v_sb)
            blk_segs = {}    # (h, j) -> {t: (p_sb, pos, o)}

            def prep_head(h):
                if h in head_sb:
                    return head_sb[h]
                qt_sb = hp.tile([128, S], BF16, tag="qt", name="qt_sb")
                kt_sb = hp.tile([128, S], BF16, tag="kt", name="kt_sb")
                v_sb = hp.tile([128, NKT, DE], BF16, tag="v", name="v_sb")
                for c0 in range(0, S, 512):
                    nc.sync.dma_start(kt_sb[:, c0:c0 + 512], kt[h, :, c0:c0 + 512])
                    nc.sync.dma_start(qt_sb[:, c0:c0 + 512], qt[h, :, c0:c0 + 512])
                for t0 in range(0, NKT, 4):
                    nc.sync.dma_start(v_sb[:, t0:t0 + 4, :], v[h, :, t0:t0 + 4, :])
                head_sb[h] = (qt_sb, kt_sb, v_sb)
                return head_sb[h]

            def emit_s(gi, grp):
                if gi % 2 == 0:
                    s_ps = ps_s.tile([128, 1536], F32, tag="sA", name="s_psA")
                else:
                    s_ps = ps_s.tile([128, 1024], F32, tag="sB", name="s_psB")
                for sg in grp["segs"]:
                    qt_sb, kt_sb, _ = prep_head(sg["h"])
                    t, j = sg["t"], sg["j"]
                    nc.tensor.matmul(
                        s_ps[:, sg["pos"]:sg["pos"] + sg["w"]],
                        kt_sb[:, t * 128:(t + 1) * 128],
                        qt_sb[:, j * QB + sg["o"]:(j + 1) * QB],
                        start=True, stop=True)
                return s_ps

            def emit_chunk_pv(h, j, b):
                # q-chunk b's P tiles (t = 0..b) all exist: one serial burst
                # of [V|1]-moving matmuls into its own PSUM bank (only one
                # open accumulation group per bank is legal), epilogue
                # immediately after.
                segs = blk_segs[(h, j)]
                _, _, v_sb = head_sb[h]
                ck = ps_c.tile([128, DE], F32, tag="ctx", name="ctx_ps")
                for t in range(b + 1):
                    p_sb, pos, o = segs[t]
                    cpos = pos + (b - 4 * j) * 128 - o
                    nc.tensor.matmul(ck, p_sb[:, cpos:cpos + 128],
                                     v_sb[:, t, :],
                                     start=(t == 0), stop=(t == b))
                recip_sb = outp.tile([128, 1], F32, tag="recip",
                                     name="recip_sb")
                nc.vector.reciprocal_approx_fast(recip_sb, ck[:, D:DE])
                ctx_sb = outp.tile([128, D], BF16, tag="ctx_out",
                                   name="ctx_sb")
                nc.vector.tensor_scalar_mul(ctx_sb, ck[:, 0:D], recip_sb)
                nc.sync.dma_start(out[h, b * 128:(b + 1) * 128, :], ctx_sb)
                if b == 4 * j + 3:
                    del blk_segs[(h, j)]

            pending = [emit_s(gi, g) for gi, g in enumerate(groups[:la])]
            for i, grp in enumerate(groups):
                if i + la < len(groups):
                    pending.append(emit_s(i + la, groups[i + la]))
                s_ps = pending.pop(0)

                p_sb = pp.tile([128, 1536], BF16, tag="p", name="p_sb")
                for a, bnd in grp["runs"]:
                    nc.scalar.activation(p_sb[:, a:bnd], s_ps[:, a:bnd], EXP,
                                         scale=SCALE)

                # zero the below-diagonal triangles, two per DVE op
                diag = sorted((sg["pos"] for sg in grp["segs"]
                               if sg["t"] >= 4 * sg["j"]))
                while diag:
                    if len(diag) >= 2:
                        p0, p1 = diag.pop(0), diag.pop(0)
                        pap = bass.AP(tensor=p_sb.tensor,
                                      offset=p_sb.offset + p0,
                                      ap=[p_sb.ap[0], [p1 - p0, 2], [1, 128]])
                        tap = bass.AP(tensor=tri_sb.tensor,
                                      offset=tri_sb.offset,
                                      ap=[tri_sb.ap[0], [0, 2], [1, 128]])
                        nc.vector.tensor_tensor(out=pap, in0=pap, in1=tap,
                                                op=MULT)
                    else:
                        p0 = diag.pop(0)
                        nc.vector.tensor_tensor(
                            out=p_sb[:, p0:p0 + 128],
                            in0=p_sb[:, p0:p0 + 128],
                            in1=tri_sb, op=MULT)

                done = []
                for sg in grp["segs"]:
                    blk_segs.setdefault((sg["h"], sg["j"]), {})[sg["t"]] = (
                        p_sb, sg["pos"], sg["o"])
                    done.extend(sg["chunks_done"])
                for h, j, b in done:
                    emit_chunk_pv(h, j, b)

    nc.compile()
    return nc


_NC_CACHE = None


def _get_nc():
    global _NC_CACHE
    if _NC_CACHE is None:
        _NC_CACHE = _build()
    return _NC_CACHE


def _prep_inputs(q, k, v):
    """Full [b,h,s,d] f32 inputs -> per-core bf16 input maps."""
    bf = ml_dtypes.bfloat16
    qf = np.asarray(q, np.float32).reshape(B * H, S, D)
    kf = np.asarray(k, np.float32).reshape(B * H, S, D)
    vf = np.asarray(v, np.float32).reshape(B * H, S, D)
    qt = qf.transpose(0, 2, 1).astype(bf)                    # [64, d, s]
    kt = kf.transpose(0, 2, 1).astype(bf)
    # [64, k_local(128), t(16), D+1] with ones in the last feature column
    vr = np.ones((B * H, 128, NKT, DE), dtype=bf)
    vr[:, :, :, :D] = vf.reshape(B * H, NKT, 128, D).transpose(0, 2, 1, 3)
    tri_np = (np.arange(128)[None, :] >= np.arange(128)[:, None]).astype(bf)
    in_maps = []
    for c in range(N_CORES):
        sl = slice(c * HPC, (c + 1) * HPC)
        in_maps.append({
            "qt": np.ascontiguousarray(qt[sl]),
            "kt": np.ascontiguousarray(kt[sl]),
            "v": np.ascontiguousarray(vr[sl]),
            "tri": tri_np,
        })
    return in_maps


def kernel(query_layer, key_layer, value_layer, attention_mask):
    """Full-input causal attention; returns [b, s, h*d] float32."""
    # attention_mask is the standard causal mask (True = masked); the kernel
    # hardcodes causal masking, so the mask tensor itself is not shipped.
    in_maps = _prep_inputs(query_layer, key_layer, value_layer)
    nc = _get_nc()
    res = run_bass_kernel_spmd(nc, in_maps, core_ids=list(range(N_CORES)))

    # [64(bh), s, d] bf16 -> out[b, s, h*D+d] f32
    o_all = np.concatenate([res.results[c]["out"] for c in range(N_CORES)],
                           axis=0)
    return np.ascontiguousarray(
        o_all.astype(np.float32).reshape(B, H, S, D).transpose(0, 2, 1, 3)
    ).reshape(B, S, H * D)


# revision 53
# speedup vs baseline: 1.1573x; 1.0320x over previous
"""Causal multi-head attention (b=2, h=32, s=2048, d=128, fp32) on 8 TRN2 NeuronCores.

Sharding: the 64 (batch, head) pairs are split 8-per-core (tensor parallel over
heads); each core runs an identical Bass/Tile kernel on its own heads.

Per-core kernel, S^T formulation with a q-chunked PV and a free softmax
denominator:
  Q^T/K^T/V/P are all bf16 (1 cycle/row matmuls at any width; halves DMA).
  The 40 causal S^T tile-segments per head are packed (widest-fit-decreasing,
  PSUM-bank-aligned) into groups alternating 1536/1024 columns, so each exp
  instruction covers ~1280 columns and the scalar engine's ~285ns/instruction
  overhead is amortized over ~109 instructions.  P^T = exp(S^T/sqrt(d)), no
  max-subtraction needed.  Diagonal-tile triangles are zeroed by DVE
  multiplies with a constant 0/1 triangle, two fused per op via a strided AP.
  PV is computed per 128-wide q-chunk with the P^T chunk as the STATIONARY
  and V extended by a ones-column as the moving operand:
      ctx[q, 0:128] , l[q] = sum_k P^T[k, q] * [V | 1][k, :]
  so the softmax denominator costs one extra matmul column (+0.8% PE) and the
  entire DVE/gpsimd l-accumulation of earlier designs disappears (power
  throttling punishes multi-engine designs: wall ~ total-engine-work/2.45).
  ctx lands q-major, so the epilogue is a [128,1] reciprocal and a 128-col
  broadcast multiply per chunk on the DVE, and chunks retire as soon as their
  diagonal tile is done - well before block end.  PSUM: 3+2 banks for the two
  S group buffers, 3 banks of [128, 3x129] ctx+l chunk tiles.
"""
import math
import sys

if '/opt/trn_rl_repo' not in sys.path:
    sys.path.insert(0, '/opt/trn_rl_repo')

import numpy as np
import ml_dtypes

import concourse.bass as bass
import concourse.tile as tile
from concourse import mybir, bacc
from concourse.bass_utils import run_bass_kernel_spmd

F32 = mybir.dt.float32
BF16 = mybir.dt.bfloat16
EXP = mybir.ActivationFunctionType.Exp
MULT = mybir.AluOpType.mult
ADD = mybir.AluOpType.add

B, H, S, D = 2, 32, 2048, 128
N_CORES = 8
HPC = (B * H) // N_CORES     # (b,h) pairs per core
QB = 512                     # q-block width (S matmul granularity)
NQB = S // QB
NKT = S // 128               # k-tiles per head
DE = D + 1                   # V extended with a ones column
SCALE = 1.0 / math.sqrt(D)


def _plan(n_heads):
    """Pack all (head, q-block, k-tile) S^T segments into PSUM groups with
    capacities alternating 1536/1024 columns (widest-fit-decreasing inside a
    block; 512-wide segs front-fill banks, narrower back-fill so closed
    groups stay contiguous).  Each segment: pos = column inside the group."""
    groups = []
    cur, banks = [], [0] * 3

    def new_group():
        nonlocal cur, banks
        nbanks = 3 if len(groups) % 2 == 0 else 2
        cur, banks = [], [0] * nbanks

    new_group()
    for h in range(n_heads):
        for j in range(NQB):
            rem = []
            for t in range(4 * j + 4):    # causal: k-tiles 0..4j+3
                o = max(t - 4 * j, 0) * 128
                rem.append(dict(h=h, j=j, t=t, w=QB - o, o=o))
            while rem:
                placed = None
                for sg in sorted(rem, key=lambda s: -s["w"]):
                    w = sg["w"]
                    rng = (range(len(banks)) if w == 512
                           else range(len(banks) - 1, -1, -1))
                    for b in rng:
                        if banks[b] + w <= 512:
                            sg["pos"] = b * 512 + banks[b]
                            banks[b] += w
                            placed = sg
                            break
                    if placed:
                        break
                if placed is None:
                    groups.append(cur)
                    new_group()
                    continue
                rem.remove(placed)
                cur.append(placed)
    if cur:
        groups.append(cur)

    # a q-chunk b of block j is complete once every tile t <= b has been
    # placed; with FFD reordering that isn't necessarily at seg (j, t=b),
    # so compute the completing segment for each chunk explicitly
    placed = {}
    for gi, grp in enumerate(groups):
        for si, sg in enumerate(grp):
            sg["chunks_done"] = []
            placed[(sg["h"], sg["j"], sg["t"])] = (gi, si, sg)
    for h in {k[0] for k in placed}:
        for j in range(NQB):
            for b in range(4 * j, 4 * j + 4):
                last = max((placed[(h, j, t)] for t in range(b + 1)),
                           key=lambda x: (x[0], x[1]))
                last[2]["chunks_done"].append((h, j, b))

    # contiguous written-column runs per group (exp must not read packing
    # holes - they hold a previous tile's stale data)
    out = []
    for grp in groups:
        ivs = sorted((sg["pos"], sg["pos"] + sg["w"]) for sg in grp)
        runs = [list(ivs[0])]
        for a, b in ivs[1:]:
            if a == runs[-1][1]:
                runs[-1][1] = b
            else:
                runs.append([a, b])
        out.append(dict(segs=grp, runs=runs))
    return out


def _build(n_heads=HPC, la=3, p_bufs=10, ctx_bufs=3):
    nc = bacc.Bacc("TRN2", target_bir_lowering=False, debug=False,
                   num_devices=N_CORES)
    qt = nc.dram_tensor("qt", [n_heads, 128, S], BF16, kind="ExternalInput")
    kt = nc.dram_tensor("kt", [n_heads, 128, S], BF16, kind="ExternalInput")
    # v layout [h, k_local(128), k_tile(16), DE]; last feature column is 1.0
    v = nc.dram_tensor("v", [n_heads, 128, NKT, DE], BF16, kind="ExternalInput")
    # tri[r, c] = 1 where c >= r else 0 (causal keep-triangle)
    tri = nc.dram_tensor("tri", [128, 128], BF16, kind="ExternalInput")
    # output is q-major per head
    out = nc.dram_tensor("out", [n_heads, S, D], BF16, kind="ExternalOutput")

    groups = _plan(n_heads)

    with tile.TileContext(nc) as tc:
        with (tc.tile_pool(name="heads", bufs=2) as hp,
              tc.tile_pool(name="consts", bufs=1) as cp,
              tc.tile_pool(name="pp", bufs=p_bufs) as pp,
              tc.tile_pool(name="outp", bufs=4) as outp,
              tc.tile_pool(name="ps_s", bufs=1, space="PSUM") as ps_s,
              tc.tile_pool(name="ps_c", bufs=ctx_bufs, space="PSUM") as ps_c):
            tri_sb = cp.tile([128, 128], BF16)
            nc.sync.dma_start(tri_sb, tri[:, :])

            head_sb = {}     # h -> (qt_sb, kt_sb, v_sb)
            blk_segs = {}    # (h, j) -> {t: (p_sb, pos, o)}

            def prep_head(h):
                if h in head_sb:
                    return head_sb[h]
                qt_sb = hp.tile([128, S], BF16, tag="qt", name="qt_sb")
                kt_sb = hp.tile([128, S], BF16, tag="kt", name="kt_sb")
                v_sb = hp.tile([128, NKT, DE], BF16, tag="v", name="v_sb")
                for c0 in range(0, S, 512):
                    nc.sync.dma_start(kt_sb[:, c0:c0 + 512], kt[h, :, c0:c0 + 512])
                    nc.sync.dma_start(qt_sb[:, c0:c0 + 512], qt[h, :, c0:c0 + 512])
                for t0 in range(0, NKT, 4):
                    nc.sync.dma_start(v_sb[:, t0:t0 + 4, :], v[h, :, t0:t0 + 4, :])
                head_sb[h] = (qt_sb, kt_sb, v_sb)
                return head_sb[h]

            def emit_s(gi, grp):
                if gi % 2 == 0:
                    s_ps = ps_s.tile([128, 1536], F32, tag="sA", name="s_psA")
                else:
                    s_ps = ps_s.tile([128, 1024], F32, tag="sB", name="s_psB")
                for sg in grp["segs"]:
                    qt_sb, kt_sb, _ = prep_head(sg["h"])
                    t, j = sg["t"], sg["j"]
                    nc.tensor.matmul(
                        s_ps[:, sg["pos"]:sg["pos"] + sg["w"]],
                        kt_sb[:, t * 128:(t + 1) * 128],
                        qt_sb[:, j * QB + sg["o"]:(j + 1) * QB],
                        start=True, stop=True)
                return s_ps

            def emit_chunk_pv(h, j, b):
                # q-chunk b's P tiles (t = 0..b) all exist: one serial burst
                # of [V|1]-moving matmuls into its own PSUM bank (only one
                # open accumulation group per bank is legal), epilogue
                # immediately after.
                segs = blk_segs[(h, j)]
                _, _, v_sb = head_sb[h]
                ck = ps_c.tile([128, DE], F32, tag="ctx", name="ctx_ps")
                for t in range(b + 1):
                    p_sb, pos, o = segs[t]
                    cpos = pos + (b - 4 * j) * 128 - o
                    nc.tensor.matmul(ck, p_sb[:, cpos:cpos + 128],
                                     v_sb[:, t, :],
                                     start=(t == 0), stop=(t == b))
                recip_sb = outp.tile([128, 1], F32, tag="recip",
                                     name="recip_sb")
                nc.vector.reciprocal_approx_fast(recip_sb, ck[:, D:DE])
                ctx_sb = outp.tile([128, D], BF16, tag="ctx_out",
                                   name="ctx_sb")
                nc.vector.tensor_scalar_mul(ctx_sb, ck[:, 0:D], recip_sb)
                nc.sync.dma_start(out[h, b * 128:(b + 1) * 128, :], ctx_sb)
                if b == 4 * j + 3:
                    del blk_segs[(h, j)]

            pending = [emit_s(gi, g) for gi, g in enumerate(groups[:la])]
            for i, grp in enumerate(groups):
                if i + la < len(groups):
                    pending.append(emit_s(i + la, groups[i + la]))
                s_ps = pending.pop(0)

                p_sb = pp.tile([128, 1536], BF16, tag="p", name="p_sb")
                for a, bnd in grp["runs"]:
                    nc.scalar.activation(p_sb[:, a:bnd], s_ps[:, a:bnd], EXP,
                                         scale=SCALE)

                # zero the below-diagonal triangles, two per DVE op
                diag = sorted((sg["pos"] for sg in grp["segs"]
                               if sg["t"] >= 4 * sg["j"]))
                while diag:
                    if len(diag) >= 2:
                        p0, p1 = diag.pop(0), diag.pop(0)
                        pap = bass.AP(tensor=p_sb.tensor,
                                      offset=p_sb.offset + p0,
                                      ap=[p_sb.ap[0], [p1 - p0, 2], [1, 128]])
                        tap = bass.AP(tensor=tri_sb.tensor,
                                      offset=tri_sb.offset,
                                      ap=[tri_sb.ap[0], [0, 2], [1, 128]])
                        nc.vector.tensor_tensor(out=pap, in0=pap, in1=tap,
                                                op=MULT)
                    else:
                        p0 = diag.pop(0)
                        nc.vector.tensor_tensor(
                            out=p_sb[:, p0:p0 + 128],
                            in0=p_sb[:, p0:p0 + 128],
                            in1=tri_sb, op=MULT)

                done = []
                for sg in grp["segs"]:
                    blk_segs.setdefault((sg["h"], sg["j"]), {})[sg["t"]] = (
                        p_sb, sg["pos"], sg["o"])
                    done.extend(sg["chunks_done"])
                for h, j, b in done:
                    emit_chunk_pv(h, j, b)

    nc.compile()
    return nc


_NC_CACHE = None


def _get_nc():
    global _NC_CACHE
    if _NC_CACHE is None:
        _NC_CACHE = _build()
    return _NC_CACHE


def _prep_inputs(q, k, v):
    """Full [b,h,s,d] f32 inputs -> per-core bf16 input maps."""
    bf = ml_dtypes.bfloat16
    qf = np.asarray(q, np.float32).reshape(B * H, S, D)
    kf = np.asarray(k, np.float32).reshape(B * H, S, D)
    vf = np.asarray(v, np.float32).reshape(B * H, S, D)
    qt = qf.transpose(0, 2, 1).astype(bf)                    # [64, d, s]
    kt = kf.transpose(0, 2, 1).astype(bf)
    # [64, k_local(128), t(16), D+1] with ones in the last feature column
    vr = np.ones((B * H, 128, NKT, DE), dtype=bf)
    vr[:, :, :, :D] = vf.reshape(B * H, NKT, 128, D).transpose(0, 2, 1, 3)
    tri_np = (np.arange(128)[None, :] >= np.arange(128)[:, None]).astype(bf)
    in_maps = []
    for c in range(N_CORES):
        sl = slice(c * HPC, (c + 1) * HPC)
        in_maps.append({
            "qt": np.ascontiguousarray(qt[sl]),
            "kt": np.ascontiguousarray(kt[sl]),
            "v": np.ascontiguousarray(vr[sl]),
            "tri": tri_np,
        })
    return in_maps


def kernel(query_layer, key_layer, value_layer, attention_mask):
    """Full-input causal attention; returns [b, s, h*d] float32."""
    # attention_mask is the standard causal mask (True = masked); the kernel
    # hardcodes causal masking, so the mask tensor itself is not shipped.
    in_maps = _prep_inputs(query_layer, key_layer, value_layer)
    nc = _get_nc()
    res = run_bass_kernel_spmd(nc, in_maps, core_ids=list(range(N_CORES)))

    # [64(bh), s, d] bf16 -> out[b, s, h*D+d] f32
    o_all = np.concatenate([res.results[c]["out"] for c in range(N_CORES)],
                           axis=0)
    return np.ascontiguousarray(
        o_all.astype(np.float32).reshape(B, H, S, D).transpose(0, 2, 1, 3)
    ).reshape(B, S, H * D)
